# revision 32
# baseline (speedup 1.0000x reference)
"""Batched LoRA Linear on 8 Trainium2 NeuronCores (Bass/Tile).

Computes, for x (32, 512, 4096), adapter_ids (32,), A_all (32, 16, 4096),
B_all (32, 4096, 16), W (4096, 4096), b (4096,):

    out = x @ W.T + b + 2.0 * ((x @ A[aid].T) @ B[aid].T)

Sharding: data-parallel over batch — 4 samples per core; W/b replicated.

Per-core device kernel v2 (883.7us measured vs the 970.6us bf16
baseline, -9.0%; tensor busy ~830us, startup ~36us DMA-bound ramp,
tail ~11us):
 - T_BLOCK=512: one sample per block, 4 t-tiles per o-pass, 4 blocks.
 - Split-k fp8: the first NF=8 of 32 k-tiles run as fp8e4 DoubleRow
   matmuls (contraction 256/instr, 2x bf16 throughput, hw-measured
   214.8ns vs 222.0ns per 512-col instr). Product-1 scales
   (x*1/8, W*8) make the fp8 partial sums land in the SAME fp32 PSUM
   accumulation group as the bf16 k-tiles - no merge pass. Measured
   rel_fro 1.897e-2 (= numpy sim prediction; gate 2e-2; NF=6 gives
   1.649e-2 at +25us if more margin is ever needed).
 - LoRA-1 fused as 16-col "rider" matmuls after each base matmul,
   reusing its stationary x-tile (hw-measured +13ns per rider vs
   ~850ns/k-tile standalone). Riders accumulate inter^T (tokens x
   rank) in a 9th psum region; a PE-transpose epilogue (4x [128,16]
   transposes via identity) yields the [17, 512] inter tile (ones row
   carries the bias through LoRA-2 as a rank-1 term). The rider bank
   is DVE-zeroed and all riders use start=False: a start=True from
   one of the 4 interleaved per-tt groups sharing the bank zeroes
   beyond its own column range (hw-observed).
 - o=0's LoRA-2 waits for the transpose epilogue (~0.5us tensor stall
   per block), then accumulates into the still-open o=0 psums before
   eviction - every pass ends uniformly via LoRA-2 stop=True. PSUM =
   7 x [128,512] ps slots + 1 rider slot = exactly 8 banks; with 4
   allocations per pass the 7-slot rotation pairs each pass's LAST
   psum with the previous pass's FIRST-evicted bank (zero-stall).
 - W streamed as bf16 k-pairs alternating sync/scalar HW-DGE queues
   (pair cadence 1.7us/queue); fp8 W (3MB) resident in SBUF.
"""

import sys
import types

import numpy as np

# ---------------------------------------------------------------- constants
P = 128
B_SZ = 32            # batch
S = 512              # seq len
D_IN = 4096
D_OUT = 4096
RANK = 16
RB = RANK + 1        # + ones row: bias rides LoRA-2 as a rank-1 term
N_CORES = 8
SPB = B_SZ // N_CORES          # samples per core = 4
T = SPB * S                    # tokens per core = 2048
KT = D_IN // P                 # 32 k-tiles
TB = 512                       # tokens per block = one sample
N_TB = T // TB                 # 4 blocks
TT = TB // P                   # 4 t-tiles per block
O_TILE = 512
N_OT = D_OUT // O_TILE         # 8 o-tiles
SCALING = 2.0

NF = 8                         # fp8 k-tiles (even); rest bf16
NB = KT - NF                   # 26 bf16 k-tiles
NPQ = NF // 2                  # fp8 DoubleRow pairs per o-pass
SX = 0.125                     # x fp8 scale; SX*SWQ == 1 (exact, pow2)
SWQ = 8.0
SAQ = 8.0                      # A fp8 scale; SX*SAQ == 1

LAST_RESULTS = None            # test harness reads exec_time_ns from here

_COMPILED = {}


def _ensure_axon_hooks_module():
    try:
        import antenv.axon_hooks  # noqa: F401
        return
    except ImportError:
        pass
    try:
        import antenv
    except ImportError:
        return
    mod = types.ModuleType("antenv.axon_hooks")
    state = {"hook": None}
    mod.set_axon_ntff_profile_hook = lambda h: state.__setitem__("hook", h)
    mod.get_axon_ntff_profile_hook = lambda: state["hook"]
    sys.modules["antenv.axon_hooks"] = mod
    antenv.axon_hooks = mod


def _build():
    import concourse.bacc as bacc
    import concourse.bass as bass
    import concourse.mybir as mybir
    import concourse.tile as tile

    f32 = mybir.dt.float32
    bf16 = mybir.dt.bfloat16
    fp8 = mybir.dt.float8e4
    DR = mybir.MatmulPerfMode.DoubleRow

    nc = bacc.Bacc("TRN2", target_bir_lowering=False, debug=False,
                   enable_asserts=False)

    xb_d = nc.dram_tensor("xb", [P, NB, T], bf16, kind="ExternalInput").ap()
    xq_d = nc.dram_tensor("xq", [P, NPQ, 2, T], fp8, kind="ExternalInput").ap()
    wb_d = nc.dram_tensor("wb", [P, NB, D_OUT], bf16, kind="ExternalInput").ap()
    wq_d = nc.dram_tensor("wq", [P, NF, D_OUT], fp8, kind="ExternalInput").ap()
    ab_d = nc.dram_tensor("ab", [P, SPB, NB, RANK], bf16,
                          kind="ExternalInput").ap()
    aq_d = nc.dram_tensor("aq", [P, SPB, NPQ, 2, RANK], fp8,
                          kind="ExternalInput").ap()
    bt_d = nc.dram_tensor("bt", [RB, SPB, D_OUT], bf16,
                          kind="ExternalInput").ap()
    id_d = nc.dram_tensor("idm", [P, P], bf16, kind="ExternalInput").ap()
    out_d = nc.dram_tensor("out", [P, T // P, D_OUT], bf16,
                           kind="ExternalOutput").ap()

    with tile.TileContext(nc) as tc:
        with (
            tc.tile_pool(name="res", bufs=1) as res_pool,
            tc.tile_pool(name="xb", bufs=2 * NB) as xb_pool,
            tc.tile_pool(name="xq", bufs=2 * NPQ) as xq_pool,
            tc.tile_pool(name="wb", bufs=8) as wb_pool,
            tc.tile_pool(name="il", bufs=2) as il_pool,
            tc.tile_pool(name="il2", bufs=2) as il2_pool,
            tc.tile_pool(name="ob", bufs=8) as out_pool,
            tc.tile_pool(name="ps", bufs=7, space="PSUM") as ps_pool,
            tc.tile_pool(name="psil", bufs=1, space="PSUM") as psil_pool,
        ):
            # ---- resident tensors ----
            wq_sb = res_pool.tile([P, NF, D_OUT], fp8, name="wq", tag="wq")
            ab_sb = res_pool.tile([P, SPB, NB, RANK], bf16, name="ab",
                                  tag="ab")
            aq_sb = res_pool.tile([P, SPB, NPQ, 2, RANK], fp8, name="aq",
                                  tag="aq")
            bt_sb = res_pool.tile([RB, SPB, D_OUT], bf16, name="bt", tag="bt")
            id_sb = res_pool.tile([P, P], bf16, name="idm", tag="idm")

            xq_tiles = [None] * N_TB
            xb_tiles = [None] * N_TB
            il2_tiles = [None] * N_TB
            pre_w = {}     # (tb, o) -> list of preloaded W pair tiles

            def emit_block0_stream():
                """Block 0 startup: ~8MB must land in ~26us (o=0 pass is
                DMA-bound from cold start). Emit everything o=0 needs in
                need order, round-robin across the two HW-DGE queues
                (sync/scalar); bf16 x even-tiles ride gpsimd (SW DGE) in
                parallel. aq leads (DR riders run from step 0); id (for
                the epilogue transpose) goes last."""
                xq_tiles[0] = [None] * NPQ
                xb_tiles[0] = [None] * NB
                HWJ = 16       # xb tiles below this index ride the HW queues
                items = [("aq", 0, 49)]
                for p in range(NPQ):
                    items += [("xq", p, 128), ("wqc", p, 128)]
                w_tiles = []
                for j in range(NB):
                    if j % 2 == 0:
                        items.append(("w", j // 2, 256))
                    if j == 5:
                        items.append(("ab", 0, 425))
                    if j < HWJ:
                        items.append(("xb", j, 128))
                items += [("idm", 0, 32)]
                qb = [0, 0]    # greedy byte balance across sync/scalar
                for kind, idx, sz in items:
                    qi = 0 if qb[0] <= qb[1] else 1
                    qb[qi] += sz
                    eng = nc.sync if qi == 0 else nc.scalar
                    if kind == "aq":
                        eng.dma_start(aq_sb[:], aq_d)
                    elif kind == "ab":
                        eng.dma_start(ab_sb[:], ab_d)
                    elif kind == "idm":
                        eng.dma_start(id_sb[:], id_d)
                    elif kind == "xq":
                        t = xq_pool.tile([P, 2, TB], fp8,
                                         name=f"xq_0_{idx}", tag="xq")
                        eng.dma_start(t[:], xq_d[:, idx, :, 0:TB])
                        xq_tiles[0][idx] = t
                    elif kind == "wqc":
                        eng.dma_start(
                            wq_sb[:, 2 * idx:2 * idx + 2, 0:O_TILE],
                            wq_d[:, 2 * idx:2 * idx + 2, 0:O_TILE])
                    elif kind == "xb":
                        t = xb_pool.tile([P, TB], bf16,
                                         name=f"xb_0_{idx}", tag="xb")
                        eng.dma_start(t[:], xb_d[:, idx, 0:TB])
                        xb_tiles[0][idx] = t
                    elif kind == "w":
                        j = 2 * idx
                        w2 = wb_pool.tile([P, 2, O_TILE], bf16,
                                          name=f"w_0_0_{j}", tag="w")
                        eng.dma_start(w2[:], wb_d[:, j:j + 2, 0:O_TILE])
                        w_tiles.append(w2)
                for j in range(HWJ, NB):
                    t = xb_pool.tile([P, TB], bf16, name=f"xb_0_{j}",
                                     tag="xb")
                    nc.gpsimd.dma_start(t[:], xb_d[:, j, 0:TB])
                    xb_tiles[0][j] = t
                nc.gpsimd.dma_start(bt_sb[:], bt_d)
                pre_w[(0, 0)] = w_tiles

            def emit_block_loads(tb):
                """x tiles for block tb>=1: stream on gpsimd during the
                previous block's ~200us of compute."""
                xq_tiles[tb] = []
                xb_tiles[tb] = []
                for p in range(NPQ):
                    t = xq_pool.tile([P, 2, TB], fp8, name=f"xq_{tb}_{p}",
                                     tag="xq")
                    nc.gpsimd.dma_start(
                        t[:], xq_d[:, p, :, tb * TB:(tb + 1) * TB])
                    xq_tiles[tb].append(t)
                for j in range(NB):
                    t = xb_pool.tile([P, TB], bf16, name=f"xb_{tb}_{j}",
                                     tag="xb")
                    nc.gpsimd.dma_start(t[:], xb_d[:, j, tb * TB:(tb + 1) * TB])
                    xb_tiles[tb].append(t)

            def emit_pass(tb, o, riders, rider_skip, bf16_first=False):
                """One o-pass: NPQ DoubleRow steps (resident fp8 W) and NB
                bf16 steps (W pairs streamed sync/scalar). DR steps lead by
                default (gives the W queues a breather at pass start);
                block0-o=0 runs bf16-first to match cold-start DMA arrival
                order. Returns the 4 psum tiles (caller finishes
                lora2/eviction)."""
                s = tb
                oc = slice(o * O_TILE, (o + 1) * O_TILE)
                psums = [ps_pool.tile([P, O_TILE], f32,
                                      name=f"ps_{tb}_{o}_{i}", tag="ps")
                         for i in range(TT)]
                state = {"r_idx": 0, "deferred": []}
                n_riders = NPQ + NB

                def dr_steps(first):
                    for p in range(NPQ):
                        xqt = xq_tiles[tb][p]
                        for tt in range(TT):
                            nc.tensor.matmul(
                                psums[tt][:],
                                xqt[:, :, tt * P:(tt + 1) * P],
                                wq_sb[:, 2 * p:2 * p + 2, oc],
                                start=(first and p == 0), stop=False,
                                perf_mode=DR)
                            if riders:
                                # start=False always: 4 interleaved
                                # accumulation groups share this bank at
                                # different column offsets, and a start=True
                                # zeroes beyond its own region (hw-observed);
                                # the bank is DVE-zeroed before the pass.
                                nc.tensor.matmul(
                                    ps_il[:, tt * RANK:(tt + 1) * RANK],
                                    xqt[:, :, tt * P:(tt + 1) * P],
                                    aq_sb[:, s, p],
                                    start=False,
                                    stop=(state["r_idx"] == n_riders - 1
                                          and not state["deferred"]),
                                    perf_mode=DR)
                        state["r_idx"] += 1

                def bf_steps(first):
                    w2 = None
                    for j in range(NB):
                        if j % 2 == 0:
                            pg = j // 2
                            if (tb, o) in pre_w:
                                w2 = pre_w[(tb, o)][pg]
                            else:
                                w2 = wb_pool.tile(
                                    [P, 2, O_TILE], bf16,
                                    name=f"w_{tb}_{o}_{j}", tag="w")
                                eng = nc.sync if pg % 2 == 0 else nc.scalar
                                eng.dma_start(w2[:], wb_d[:, j:j + 2, oc])
                            # block 0: stream the next wq o-chunk during the
                            # previous pass's pair stream
                            if tb == 0 and o < N_OT - 1 and pg == 2:
                                oc2 = slice((o + 1) * O_TILE,
                                            (o + 2) * O_TILE)
                                nc.scalar.dma_start(wq_sb[:, :, oc2],
                                                    wq_d[:, :, oc2])
                        for tt in range(TT):
                            nc.tensor.matmul(
                                psums[tt][:],
                                xb_tiles[tb][j][:, tt * P:(tt + 1) * P],
                                w2[:, j % 2],
                                start=(first and j == 0), stop=False)
                            if riders:
                                if rider_skip and j < rider_skip:
                                    if tt == 0:
                                        state["deferred"].append(j)
                                    continue
                                nc.tensor.matmul(
                                    ps_il[:, tt * RANK:(tt + 1) * RANK],
                                    xb_tiles[tb][j][:, tt * P:(tt + 1) * P],
                                    ab_sb[:, s, j],
                                    start=False,
                                    stop=(state["r_idx"] == n_riders - 1
                                          and not state["deferred"]))
                        state["r_idx"] += 1

                if bf16_first:
                    bf_steps(True)
                    dr_steps(False)
                else:
                    dr_steps(True)
                    bf_steps(False)
                deferred = state["deferred"]
                # cleanup riders whose ab tile hadn't landed yet (block 0
                # start): re-load the stationary (costs an exposed LS each)
                for di, j in enumerate(deferred):
                    for tt in range(TT):
                        nc.tensor.matmul(
                            ps_il[:, tt * RANK:(tt + 1) * RANK],
                            xb_tiles[tb][j][:, tt * P:(tt + 1) * P],
                            ab_sb[:, s, j],
                            start=False, stop=(di == len(deferred) - 1))
                return psums

            def emit_lora2(tb, o, psums):
                s = tb
                oc = slice(o * O_TILE, (o + 1) * O_TILE)
                for tt in range(TT):
                    nc.tensor.matmul(
                        psums[tt][:],
                        il2_tiles[tb][:, tt * P:(tt + 1) * P],
                        bt_sb[:, s, oc],
                        start=False, stop=True)

            def emit_evict(tb, o, psums, final=False):
                for tt in range(TT):
                    o_t = out_pool.tile([P, O_TILE], bf16,
                                        name=f"o_{tb}_{o}_{tt}", tag="o")
                    if final and tt % 2 == 1:
                        # tail: split the last evictions across ACT + DVE
                        nc.scalar.copy(o_t[:], psums[tt][:])
                        nc.sync.dma_start(
                            out_d[:, tb * TT + tt,
                                  o * O_TILE:(o + 1) * O_TILE], o_t[:])
                    else:
                        nc.vector.tensor_copy(o_t[:], psums[tt][:])
                        nc.scalar.dma_start(
                            out_d[:, tb * TT + tt,
                                  o * O_TILE:(o + 1) * O_TILE], o_t[:])

            emit_block0_stream()

            for tb in range(N_TB):
                # rider psum: [tokens, tt*rank] — 9th psum region (1 bank)
                ps_il = psil_pool.tile([P, TT * RANK], f32,
                                       name=f"psil_{tb}", tag="psil")
                nc.vector.memset(ps_il[:], 0.0)

                # ---- o=0: base + riders; lora2 after the epilogue ----
                psums0 = emit_pass(tb, 0, riders=True,
                                   rider_skip=(8 if tb == 0 else 0))
                # LoRA-1 epilogue: psum_il -> sbuf -> PE-transpose (via
                # identity) -> [17, 512] inter tile with ones row (bias)
                il_sb = il_pool.tile([P, TT * RANK], bf16,
                                     name=f"il_{tb}", tag="il")
                nc.vector.tensor_copy(il_sb[:], ps_il[:])
                tr_ps = psil_pool.tile([RANK, TT, P], bf16,
                                       name=f"tr_{tb}", tag="psil")
                for tt in range(TT):
                    nc.tensor.transpose(
                        tr_ps[:, tt, :],
                        il_sb[:, tt * RANK:(tt + 1) * RANK], id_sb[:])
                il2 = il2_pool.tile([RB, TB], bf16, name=f"il2_{tb}",
                                    tag="il2")
                nc.vector.memset(il2[:], 1.0)
                nc.vector.tensor_copy(il2[0:RANK, :], tr_ps[:])
                il2_tiles[tb] = il2

                emit_lora2(tb, 0, psums0)
                emit_evict(tb, 0, psums0)

                for o in range(1, N_OT):
                    psums = emit_pass(tb, o, riders=False, rider_skip=0)
                    if o == 1 and tb + 1 < N_TB:
                        emit_block_loads(tb + 1)
                    emit_lora2(tb, o, psums)
                    final = (tb == N_TB - 1 and o == N_OT - 1)
                    emit_evict(tb, o, psums, final=final)

    nc.compile()
    return nc


def _get_compiled():
    if "nc" not in _COMPILED:
        _COMPILED["nc"] = _build()
    return _COMPILED["nc"]


def kernel(x, adapter_ids, A_all, B_all, W, b):
    global LAST_RESULTS
    _ensure_axon_hooks_module()
    from concourse.bass_utils import run_bass_kernel_spmd
    from ml_dtypes import bfloat16, float8_e4m3fn

    x = np.asarray(x, dtype=np.float32)
    adapter_ids = np.asarray(adapter_ids)
    A_all = np.asarray(A_all, dtype=np.float32)
    B_all = np.asarray(B_all, dtype=np.float32)
    W = np.asarray(W, dtype=np.float32)
    b = np.asarray(b, dtype=np.float32)

    nc = _get_compiled()

    # ---- host-side layout prep (gather/scale/cast/transpose only) ----
    # W^T tiles: [p, k, o] = W[o, k*128+p]
    wt = np.ascontiguousarray(
        W.T.reshape(KT, P, D_OUT).transpose(1, 0, 2))        # (P, KT, D_OUT)
    A_batch = A_all[adapter_ids]                             # (B, R, D_IN)
    B_batch = B_all[adapter_ids] * SCALING                   # (B, D_OUT, R)

    idm = np.eye(P, dtype=np.float32).astype(bfloat16)

    in_maps = []
    for c in range(N_CORES):
        # stagger each core's o-axis so the SPMD cores don't all stream
        # the same W bytes at the same instant
        sh = (c % N_OT) * O_TILE
        wt_c = np.roll(wt, -sh, axis=2)
        wq_np = np.ascontiguousarray(wt_c[:, :NF, :] * SWQ).astype(
            float8_e4m3fn)
        wb_np = np.ascontiguousarray(wt_c[:, NF:, :]).astype(bfloat16)

        xs = x[c * SPB:(c + 1) * SPB].reshape(T, D_IN)
        xt = xs.reshape(T, KT, P).transpose(2, 1, 0)          # (P, KT, T)
        xq_np = np.ascontiguousarray(
            (xt[:, :NF, :] * SX).reshape(P, NPQ, 2, T)).astype(float8_e4m3fn)
        xb_np = np.ascontiguousarray(xt[:, NF:, :]).astype(bfloat16)

        A_c = A_batch[c * SPB:(c + 1) * SPB]                  # (SPB, R, D_IN)
        at = A_c.reshape(SPB, RANK, KT, P).transpose(3, 0, 2, 1)
        # (P, SPB, KT, R)
        aq_np = np.ascontiguousarray(
            (at[:, :, :NF, :] * SAQ).reshape(P, SPB, NPQ, 2, RANK)).astype(
            float8_e4m3fn)
        ab_np = np.ascontiguousarray(at[:, :, NF:, :]).astype(bfloat16)

        B_c = B_batch[c * SPB:(c + 1) * SPB]                  # (SPB, D_OUT, R)
        bt_base = np.roll(B_c.transpose(2, 0, 1), -sh, axis=2)  # (R, SPB, DO)
        bias_row = np.broadcast_to(np.roll(b, -sh), (1, SPB, D_OUT))
        bt_np = np.ascontiguousarray(
            np.concatenate([bt_base, bias_row], axis=0)).astype(bfloat16)

        in_maps.append({
            "xb": xb_np, "xq": xq_np, "wb": wb_np, "wq": wq_np,
            "ab": ab_np, "aq": aq_np, "bt": bt_np, "idm": idm,
        })

    res = run_bass_kernel_spmd(nc, in_maps, core_ids=list(range(N_CORES)))
    LAST_RESULTS = res

    out = np.empty((B_SZ, S, D_OUT), dtype=np.float32)
    for c in range(N_CORES):
        sh = (c % N_OT) * O_TILE
        oc = np.roll(res.results[c]["out"].astype(np.float32), sh, axis=2)
        out[c * SPB:(c + 1) * SPB] = (
            oc.transpose(1, 0, 2).reshape(T, D_OUT).reshape(SPB, S, D_OUT))
    return out


# revision 33
# speedup vs baseline: 1.1827x; 1.1827x over previous
"""Batched LoRA Linear on 8 Trainium2 NeuronCores (Bass/Tile).

Computes, for x (32, 512, 4096), adapter_ids (32,), A_all (32, 16, 4096),
B_all (32, 4096, 16), W (4096, 4096), b (4096,):

    out = x @ W.T + b + 2.0 * ((x @ A[aid].T) @ B[aid].T)

Sharding: data-parallel over batch — 4 samples per core; W/b replicated.

Per-core device kernel v2 (883.7us measured vs the 970.6us bf16
baseline, -9.0%; tensor busy ~830us, startup ~36us DMA-bound ramp,
tail ~11us):
 - T_BLOCK=512: one sample per block, 4 t-tiles per o-pass, 4 blocks.
 - Split-k fp8: the first NF=8 of 32 k-tiles run as fp8e4 DoubleRow
   matmuls (contraction 256/instr, 2x bf16 throughput, hw-measured
   214.8ns vs 222.0ns per 512-col instr). Product-1 scales
   (x*1/8, W*8) make the fp8 partial sums land in the SAME fp32 PSUM
   accumulation group as the bf16 k-tiles - no merge pass. Measured
   rel_fro 1.897e-2 (= numpy sim prediction; gate 2e-2; NF=6 gives
   1.649e-2 at +25us if more margin is ever needed).
 - LoRA-1 fused as 16-col "rider" matmuls after each base matmul,
   reusing its stationary x-tile (hw-measured +13ns per rider vs
   ~850ns/k-tile standalone). Riders accumulate inter^T (tokens x
   rank) in a 9th psum region; a PE-transpose epilogue (4x [128,16]
   transposes via identity) yields the [17, 512] inter tile (ones row
   carries the bias through LoRA-2 as a rank-1 term). The rider bank
   is DVE-zeroed and all riders use start=False: a start=True from
   one of the 4 interleaved per-tt groups sharing the bank zeroes
   beyond its own column range (hw-observed).
 - o=0's LoRA-2 waits for the transpose epilogue (~0.5us tensor stall
   per block), then accumulates into the still-open o=0 psums before
   eviction - every pass ends uniformly via LoRA-2 stop=True. PSUM =
   7 x [128,512] ps slots + 1 rider slot = exactly 8 banks; with 4
   allocations per pass the 7-slot rotation pairs each pass's LAST
   psum with the previous pass's FIRST-evicted bank (zero-stall).
 - W streamed as bf16 k-pairs alternating sync/scalar HW-DGE queues
   (pair cadence 1.7us/queue); fp8 W (3MB) resident in SBUF.
"""

import sys
import types

import numpy as np

# ---------------------------------------------------------------- constants
P = 128
B_SZ = 32            # batch
S = 512              # seq len
D_IN = 4096
D_OUT = 4096
RANK = 16
RB = RANK + 1        # + ones row: bias rides LoRA-2 as a rank-1 term
N_CORES = 8
SPB = B_SZ // N_CORES          # samples per core = 4
T = SPB * S                    # tokens per core = 2048
KT = D_IN // P                 # 32 k-tiles
TB = 512                       # tokens per block = one sample
N_TB = T // TB                 # 4 blocks
TT = TB // P                   # 4 t-tiles per block
O_TILE = 512
N_OT = D_OUT // O_TILE         # 8 o-tiles
SCALING = 2.0

NF = 8                         # fp8 k-tiles (even); rest bf16
NB = KT - NF                   # 26 bf16 k-tiles
NPQ = NF // 2                  # fp8 DoubleRow pairs per o-pass
SX = 0.125                     # x fp8 scale; SX*SWQ == 1 (exact, pow2)
SWQ = 8.0
SAQ = 8.0                      # A fp8 scale; SX*SAQ == 1

LAST_RESULTS = None            # test harness reads exec_time_ns from here

_COMPILED = {}


def _ensure_axon_hooks_module():
    try:
        import antenv.axon_hooks  # noqa: F401
        return
    except ImportError:
        pass
    try:
        import antenv
    except ImportError:
        return
    mod = types.ModuleType("antenv.axon_hooks")
    state = {"hook": None}
    mod.set_axon_ntff_profile_hook = lambda h: state.__setitem__("hook", h)
    mod.get_axon_ntff_profile_hook = lambda: state["hook"]
    sys.modules["antenv.axon_hooks"] = mod
    antenv.axon_hooks = mod


def _build():
    import concourse.bacc as bacc
    import concourse.bass as bass
    import concourse.mybir as mybir
    import concourse.tile as tile

    f32 = mybir.dt.float32
    bf16 = mybir.dt.bfloat16
    fp8 = mybir.dt.float8e4
    DR = mybir.MatmulPerfMode.DoubleRow

    nc = bacc.Bacc("TRN2", target_bir_lowering=False, debug=False,
                   enable_asserts=False)

    xb_d = nc.dram_tensor("xb", [P, NB, T], bf16, kind="ExternalInput").ap()
    xq_d = nc.dram_tensor("xq", [P, NPQ, 2, T], fp8, kind="ExternalInput").ap()
    wb_d = nc.dram_tensor("wb", [P, NB, D_OUT], bf16, kind="ExternalInput").ap()
    wq_d = nc.dram_tensor("wq", [P, NF, D_OUT], fp8, kind="ExternalInput").ap()
    ab_d = nc.dram_tensor("ab", [P, SPB, NB, RANK], bf16,
                          kind="ExternalInput").ap()
    aq_d = nc.dram_tensor("aq", [P, SPB, NPQ, 2, RANK], fp8,
                          kind="ExternalInput").ap()
    bt_d = nc.dram_tensor("bt", [RB, SPB, D_OUT], bf16,
                          kind="ExternalInput").ap()
    id_d = nc.dram_tensor("idm", [P, P], bf16, kind="ExternalInput").ap()
    out_d = nc.dram_tensor("out", [P, T // P, D_OUT], bf16,
                           kind="ExternalOutput").ap()

    with tile.TileContext(nc) as tc:
        with (
            tc.tile_pool(name="res", bufs=1) as res_pool,
            tc.tile_pool(name="xb", bufs=2 * NB) as xb_pool,
            tc.tile_pool(name="xq", bufs=2 * NPQ) as xq_pool,
            tc.tile_pool(name="wb", bufs=8) as wb_pool,
            tc.tile_pool(name="il", bufs=2) as il_pool,
            tc.tile_pool(name="il2", bufs=2) as il2_pool,
            tc.tile_pool(name="ob", bufs=8) as out_pool,
            tc.tile_pool(name="ps", bufs=7, space="PSUM") as ps_pool,
            tc.tile_pool(name="psil", bufs=1, space="PSUM") as psil_pool,
        ):
            # ---- resident tensors ----
            wq_sb = res_pool.tile([P, NF, D_OUT], fp8, name="wq", tag="wq")
            ab_sb = res_pool.tile([P, SPB, NB, RANK], bf16, name="ab",
                                  tag="ab")
            aq_sb = res_pool.tile([P, SPB, NPQ, 2, RANK], fp8, name="aq",
                                  tag="aq")
            bt_sb = res_pool.tile([RB, SPB, D_OUT], bf16, name="bt", tag="bt")
            id_sb = res_pool.tile([P, P], bf16, name="idm", tag="idm")

            xq_tiles = [None] * N_TB
            xb_tiles = [None] * N_TB
            il2_tiles = [None] * N_TB
            pre_w = {}     # (tb, o) -> list of preloaded W pair tiles

            def emit_block0_stream():
                """Block 0 startup: ~8MB must land in ~26us (o=0 pass is
                DMA-bound from cold start). Emit everything o=0 needs in
                need order, round-robin across the two HW-DGE queues
                (sync/scalar); bf16 x even-tiles ride gpsimd (SW DGE) in
                parallel. aq leads (DR riders run from step 0); id (for
                the epilogue transpose) goes last."""
                xq_tiles[0] = [None] * NPQ
                xb_tiles[0] = [None] * NB
                HWJ = 16       # xb tiles below this index ride the HW queues
                items = [("aq", 0, 49)]
                for p in range(NPQ):
                    items += [("xq", p, 128), ("wqc", p, 128)]
                w_tiles = []
                for j in range(NB):
                    if j % 2 == 0:
                        items.append(("w", j // 2, 256))
                    if j == 5:
                        items.append(("ab", 0, 425))
                    if j < HWJ:
                        items.append(("xb", j, 128))
                items += [("idm", 0, 32)]
                qb = [0, 0]    # greedy byte balance across sync/scalar
                for kind, idx, sz in items:
                    qi = 0 if qb[0] <= qb[1] else 1
                    qb[qi] += sz
                    eng = nc.sync if qi == 0 else nc.scalar
                    if kind == "aq":
                        eng.dma_start(aq_sb[:], aq_d)
                    elif kind == "ab":
                        eng.dma_start(ab_sb[:], ab_d)
                    elif kind == "idm":
                        eng.dma_start(id_sb[:], id_d)
                    elif kind == "xq":
                        t = xq_pool.tile([P, 2, TB], fp8,
                                         name=f"xq_0_{idx}", tag="xq")
                        eng.dma_start(t[:], xq_d[:, idx, :, 0:TB])
                        xq_tiles[0][idx] = t
                    elif kind == "wqc":
                        eng.dma_start(
                            wq_sb[:, 2 * idx:2 * idx + 2, 0:O_TILE],
                            wq_d[:, 2 * idx:2 * idx + 2, 0:O_TILE])
                    elif kind == "xb":
                        t = xb_pool.tile([P, TB], bf16,
                                         name=f"xb_0_{idx}", tag="xb")
                        eng.dma_start(t[:], xb_d[:, idx, 0:TB])
                        xb_tiles[0][idx] = t
                    elif kind == "w":
                        j = 2 * idx
                        w2 = wb_pool.tile([P, 2, O_TILE], bf16,
                                          name=f"w_0_0_{j}", tag="w")
                        eng.dma_start(w2[:], wb_d[:, j:j + 2, 0:O_TILE])
                        w_tiles.append(w2)
                for j in range(HWJ, NB):
                    t = xb_pool.tile([P, TB], bf16, name=f"xb_0_{j}",
                                     tag="xb")
                    nc.gpsimd.dma_start(t[:], xb_d[:, j, 0:TB])
                    xb_tiles[0][j] = t
                nc.gpsimd.dma_start(bt_sb[:], bt_d)
                pre_w[(0, 0)] = w_tiles

            def emit_block_loads(tb):
                """x tiles for block tb>=1: stream on gpsimd during the
                previous block's ~200us of compute."""
                xq_tiles[tb] = []
                xb_tiles[tb] = []
                for p in range(NPQ):
                    t = xq_pool.tile([P, 2, TB], fp8, name=f"xq_{tb}_{p}",
                                     tag="xq")
                    nc.gpsimd.dma_start(
                        t[:], xq_d[:, p, :, tb * TB:(tb + 1) * TB])
                    xq_tiles[tb].append(t)
                for j in range(NB):
                    t = xb_pool.tile([P, TB], bf16, name=f"xb_{tb}_{j}",
                                     tag="xb")
                    nc.gpsimd.dma_start(t[:], xb_d[:, j, tb * TB:(tb + 1) * TB])
                    xb_tiles[tb].append(t)

            def emit_pass(tb, o, riders, rider_skip, bf16_first=False):
                """One o-pass: NPQ DoubleRow steps (resident fp8 W) and NB
                bf16 steps (W pairs streamed sync/scalar). DR steps lead by
                default (gives the W queues a breather at pass start);
                block0-o=0 runs bf16-first to match cold-start DMA arrival
                order. Returns the 4 psum tiles (caller finishes
                lora2/eviction)."""
                s = tb
                oc = slice(o * O_TILE, (o + 1) * O_TILE)
                psums = [ps_pool.tile([P, O_TILE], f32,
                                      name=f"ps_{tb}_{o}_{i}", tag="ps")
                         for i in range(TT)]
                state = {"r_idx": 0, "deferred": []}
                n_riders = NPQ + NB

                def dr_steps(first):
                    for p in range(NPQ):
                        xqt = xq_tiles[tb][p]
                        for tt in range(TT):
                            nc.tensor.matmul(
                                psums[tt][:],
                                xqt[:, :, tt * P:(tt + 1) * P],
                                wq_sb[:, 2 * p:2 * p + 2, oc],
                                start=(first and p == 0), stop=False,
                                perf_mode=DR)
                            if riders:
                                # start=False always: 4 interleaved
                                # accumulation groups share this bank at
                                # different column offsets, and a start=True
                                # zeroes beyond its own region (hw-observed);
                                # the bank is DVE-zeroed before the pass.
                                nc.tensor.matmul(
                                    ps_il[:, tt * RANK:(tt + 1) * RANK],
                                    xqt[:, :, tt * P:(tt + 1) * P],
                                    aq_sb[:, s, p],
                                    start=False,
                                    stop=(state["r_idx"] == n_riders - 1
                                          and not state["deferred"]),
                                    perf_mode=DR)
                        state["r_idx"] += 1

                def bf_steps(first):
                    w2 = None
                    for j in range(NB):
                        if j % 2 == 0:
                            pg = j // 2
                            if (tb, o) in pre_w:
                                w2 = pre_w[(tb, o)][pg]
                            else:
                                w2 = wb_pool.tile(
                                    [P, 2, O_TILE], bf16,
                                    name=f"w_{tb}_{o}_{j}", tag="w")
                                eng = nc.sync if pg % 2 == 0 else nc.scalar
                                eng.dma_start(w2[:], wb_d[:, j:j + 2, oc])
                            # block 0: stream the next wq o-chunk late in the
                            # previous pass's pair stream (early injection
                            # delayed that pass's own pairs on the cold
                            # scalar queue - 4-5us gaps at the o=1/o=2
                            # boundaries in the trace)
                            if tb == 0 and o < N_OT - 1 and pg == 8:
                                oc2 = slice((o + 1) * O_TILE,
                                            (o + 2) * O_TILE)
                                nc.scalar.dma_start(wq_sb[:, :, oc2],
                                                    wq_d[:, :, oc2])
                        for tt in range(TT):
                            nc.tensor.matmul(
                                psums[tt][:],
                                xb_tiles[tb][j][:, tt * P:(tt + 1) * P],
                                w2[:, j % 2],
                                start=(first and j == 0), stop=False)
                            if riders:
                                if rider_skip and j < rider_skip:
                                    if tt == 0:
                                        state["deferred"].append(j)
                                    continue
                                nc.tensor.matmul(
                                    ps_il[:, tt * RANK:(tt + 1) * RANK],
                                    xb_tiles[tb][j][:, tt * P:(tt + 1) * P],
                                    ab_sb[:, s, j],
                                    start=False,
                                    stop=(state["r_idx"] == n_riders - 1
                                          and not state["deferred"]))
                        state["r_idx"] += 1

                if bf16_first:
                    bf_steps(True)
                    dr_steps(False)
                else:
                    dr_steps(True)
                    bf_steps(False)
                deferred = state["deferred"]
                # cleanup riders whose ab tile hadn't landed yet (block 0
                # start): re-load the stationary (costs an exposed LS each)
                for di, j in enumerate(deferred):
                    for tt in range(TT):
                        nc.tensor.matmul(
                            ps_il[:, tt * RANK:(tt + 1) * RANK],
                            xb_tiles[tb][j][:, tt * P:(tt + 1) * P],
                            ab_sb[:, s, j],
                            start=False, stop=(di == len(deferred) - 1))
                return psums

            def emit_lora2(tb, o, psums):
                s = tb
                oc = slice(o * O_TILE, (o + 1) * O_TILE)
                for tt in range(TT):
                    nc.tensor.matmul(
                        psums[tt][:],
                        il2_tiles[tb][:, tt * P:(tt + 1) * P],
                        bt_sb[:, s, oc],
                        start=False, stop=True)

            def emit_evict(tb, o, psums, final=False):
                for tt in range(TT):
                    o_t = out_pool.tile([P, O_TILE], bf16,
                                        name=f"o_{tb}_{o}_{tt}", tag="o")
                    if final and tt % 2 == 1:
                        # tail: split the last evictions across ACT + DVE
                        nc.scalar.copy(o_t[:], psums[tt][:])
                        nc.sync.dma_start(
                            out_d[:, tb * TT + tt,
                                  o * O_TILE:(o + 1) * O_TILE], o_t[:])
                    else:
                        nc.vector.tensor_copy(o_t[:], psums[tt][:])
                        nc.scalar.dma_start(
                            out_d[:, tb * TT + tt,
                                  o * O_TILE:(o + 1) * O_TILE], o_t[:])

            emit_block0_stream()

            for tb in range(N_TB):
                # rider psum: [tokens, tt*rank] — 9th psum region (1 bank)
                ps_il = psil_pool.tile([P, TT * RANK], f32,
                                       name=f"psil_{tb}", tag="psil")
                nc.vector.memset(ps_il[:], 0.0)

                # ---- o=0: base + riders; lora2 after the epilogue ----
                psums0 = emit_pass(tb, 0, riders=True,
                                   rider_skip=(8 if tb == 0 else 0))
                # LoRA-1 epilogue: psum_il -> sbuf -> PE-transpose (via
                # identity) -> [17, 512] inter tile with ones row (bias)
                il_sb = il_pool.tile([P, TT * RANK], bf16,
                                     name=f"il_{tb}", tag="il")
                nc.vector.tensor_copy(il_sb[:], ps_il[:])
                tr_ps = psil_pool.tile([RANK, TT, P], bf16,
                                       name=f"tr_{tb}", tag="psil")
                for tt in range(TT):
                    nc.tensor.transpose(
                        tr_ps[:, tt, :],
                        il_sb[:, tt * RANK:(tt + 1) * RANK], id_sb[:])
                il2 = il2_pool.tile([RB, TB], bf16, name=f"il2_{tb}",
                                    tag="il2")
                nc.vector.memset(il2[:], 1.0)
                nc.vector.tensor_copy(il2[0:RANK, :], tr_ps[:])
                il2_tiles[tb] = il2

                emit_lora2(tb, 0, psums0)
                emit_evict(tb, 0, psums0)

                for o in range(1, N_OT):
                    psums = emit_pass(tb, o, riders=False, rider_skip=0)
                    if o == 1 and tb + 1 < N_TB:
                        emit_block_loads(tb + 1)
                    emit_lora2(tb, o, psums)
                    final = (tb == N_TB - 1 and o == N_OT - 1)
                    emit_evict(tb, o, psums, final=final)

    nc.compile()
    return nc


def _get_compiled():
    if "nc" not in _COMPILED:
        _COMPILED["nc"] = _build()
    return _COMPILED["nc"]


def kernel(x, adapter_ids, A_all, B_all, W, b):
    global LAST_RESULTS
    _ensure_axon_hooks_module()
    from concourse.bass_utils import run_bass_kernel_spmd
    from ml_dtypes import bfloat16, float8_e4m3fn

    x = np.asarray(x, dtype=np.float32)
    adapter_ids = np.asarray(adapter_ids)
    A_all = np.asarray(A_all, dtype=np.float32)
    B_all = np.asarray(B_all, dtype=np.float32)
    W = np.asarray(W, dtype=np.float32)
    b = np.asarray(b, dtype=np.float32)

    nc = _get_compiled()

    # ---- host-side layout prep (gather/scale/cast/transpose only) ----
    # W^T tiles: [p, k, o] = W[o, k*128+p]
    wt = np.ascontiguousarray(
        W.T.reshape(KT, P, D_OUT).transpose(1, 0, 2))        # (P, KT, D_OUT)
    A_batch = A_all[adapter_ids]                             # (B, R, D_IN)
    B_batch = B_all[adapter_ids] * SCALING                   # (B, D_OUT, R)

    idm = np.eye(P, dtype=np.float32).astype(bfloat16)

    in_maps = []
    for c in range(N_CORES):
        # stagger each core's o-axis so the SPMD cores don't all stream
        # the same W bytes at the same instant
        sh = (c % N_OT) * O_TILE
        wt_c = np.roll(wt, -sh, axis=2)
        wq_np = np.ascontiguousarray(wt_c[:, :NF, :] * SWQ).astype(
            float8_e4m3fn)
        wb_np = np.ascontiguousarray(wt_c[:, NF:, :]).astype(bfloat16)

        xs = x[c * SPB:(c + 1) * SPB].reshape(T, D_IN)
        xt = xs.reshape(T, KT, P).transpose(2, 1, 0)          # (P, KT, T)
        xq_np = np.ascontiguousarray(
            (xt[:, :NF, :] * SX).reshape(P, NPQ, 2, T)).astype(float8_e4m3fn)
        xb_np = np.ascontiguousarray(xt[:, NF:, :]).astype(bfloat16)

        A_c = A_batch[c * SPB:(c + 1) * SPB]                  # (SPB, R, D_IN)
        at = A_c.reshape(SPB, RANK, KT, P).transpose(3, 0, 2, 1)
        # (P, SPB, KT, R)
        aq_np = np.ascontiguousarray(
            (at[:, :, :NF, :] * SAQ).reshape(P, SPB, NPQ, 2, RANK)).astype(
            float8_e4m3fn)
        ab_np = np.ascontiguousarray(at[:, :, NF:, :]).astype(bfloat16)

        B_c = B_batch[c * SPB:(c + 1) * SPB]                  # (SPB, D_OUT, R)
        bt_base = np.roll(B_c.transpose(2, 0, 1), -sh, axis=2)  # (R, SPB, DO)
        bias_row = np.broadcast_to(np.roll(b, -sh), (1, SPB, D_OUT))
        bt_np = np.ascontiguousarray(
            np.concatenate([bt_base, bias_row], axis=0)).astype(bfloat16)

        in_maps.append({
            "xb": xb_np, "xq": xq_np, "wb": wb_np, "wq": wq_np,
            "ab": ab_np, "aq": aq_np, "bt": bt_np, "idm": idm,
        })

    res = run_bass_kernel_spmd(nc, in_maps, core_ids=list(range(N_CORES)))
    LAST_RESULTS = res

    out = np.empty((B_SZ, S, D_OUT), dtype=np.float32)
    for c in range(N_CORES):
        sh = (c % N_OT) * O_TILE
        oc = np.roll(res.results[c]["out"].astype(np.float32), sh, axis=2)
        out[c * SPB:(c + 1) * SPB] = (
            oc.transpose(1, 0, 2).reshape(T, D_OUT).reshape(SPB, S, D_OUT))
    return out


# revision 38
# speedup vs baseline: 1.1828x; 1.0001x over previous
"""Batched LoRA Linear on 8 Trainium2 NeuronCores (Bass/Tile).

Computes, for x (32, 512, 4096), adapter_ids (32,), A_all (32, 16, 4096),
B_all (32, 4096, 16), W (4096, 4096), b (4096,):

    out = x @ W.T + b + 2.0 * ((x @ A[aid].T) @ B[aid].T)

Sharding: data-parallel over batch — 4 samples per core; W/b replicated.

Per-core device kernel v2 (883.7us measured vs the 970.6us bf16
baseline, -9.0%; tensor busy ~830us, startup ~36us DMA-bound ramp,
tail ~11us):
 - T_BLOCK=512: one sample per block, 4 t-tiles per o-pass, 4 blocks.
 - Split-k fp8: the first NF=8 of 32 k-tiles run as fp8e4 DoubleRow
   matmuls (contraction 256/instr, 2x bf16 throughput, hw-measured
   214.8ns vs 222.0ns per 512-col instr). Product-1 scales
   (x*1/8, W*8) make the fp8 partial sums land in the SAME fp32 PSUM
   accumulation group as the bf16 k-tiles - no merge pass. Measured
   rel_fro 1.897e-2 (= numpy sim prediction; gate 2e-2; NF=6 gives
   1.649e-2 at +25us if more margin is ever needed).
 - LoRA-1 fused as 16-col "rider" matmuls after each base matmul,
   reusing its stationary x-tile (hw-measured +13ns per rider vs
   ~850ns/k-tile standalone). Riders accumulate inter^T (tokens x
   rank) in a 9th psum region; a PE-transpose epilogue (4x [128,16]
   transposes via identity) yields the [17, 512] inter tile (ones row
   carries the bias through LoRA-2 as a rank-1 term). The rider bank
   is DVE-zeroed and all riders use start=False: a start=True from
   one of the 4 interleaved per-tt groups sharing the bank zeroes
   beyond its own column range (hw-observed).
 - o=0's LoRA-2 waits for the transpose epilogue (~0.5us tensor stall
   per block), then accumulates into the still-open o=0 psums before
   eviction - every pass ends uniformly via LoRA-2 stop=True. PSUM =
   7 x [128,512] ps slots + 1 rider slot = exactly 8 banks; with 4
   allocations per pass the 7-slot rotation pairs each pass's LAST
   psum with the previous pass's FIRST-evicted bank (zero-stall).
 - W streamed as bf16 k-pairs alternating sync/scalar HW-DGE queues
   (pair cadence 1.7us/queue); fp8 W (3MB) resident in SBUF.
"""

import sys
import types

import numpy as np

# ---------------------------------------------------------------- constants
P = 128
B_SZ = 32            # batch
S = 512              # seq len
D_IN = 4096
D_OUT = 4096
RANK = 16
RB = RANK + 1        # + ones row: bias rides LoRA-2 as a rank-1 term
N_CORES = 8
SPB = B_SZ // N_CORES          # samples per core = 4
T = SPB * S                    # tokens per core = 2048
KT = D_IN // P                 # 32 k-tiles
TB = 512                       # tokens per block = one sample
N_TB = T // TB                 # 4 blocks
TT = TB // P                   # 4 t-tiles per block
O_TILE = 512
N_OT = D_OUT // O_TILE         # 8 o-tiles
SCALING = 2.0

NF = 8                         # fp8 k-tiles (even); rest bf16
NB = KT - NF                   # 26 bf16 k-tiles
NPQ = NF // 2                  # fp8 DoubleRow pairs per o-pass
SX = 0.125                     # x fp8 scale; SX*SWQ == 1 (exact, pow2)
SWQ = 8.0
SAQ = 8.0                      # A fp8 scale; SX*SAQ == 1

LAST_RESULTS = None            # test harness reads exec_time_ns from here

_COMPILED = {}


def _ensure_axon_hooks_module():
    try:
        import antenv.axon_hooks  # noqa: F401
        return
    except ImportError:
        pass
    try:
        import antenv
    except ImportError:
        return
    mod = types.ModuleType("antenv.axon_hooks")
    state = {"hook": None}
    mod.set_axon_ntff_profile_hook = lambda h: state.__setitem__("hook", h)
    mod.get_axon_ntff_profile_hook = lambda: state["hook"]
    sys.modules["antenv.axon_hooks"] = mod
    antenv.axon_hooks = mod


def _build():
    import concourse.bacc as bacc
    import concourse.bass as bass
    import concourse.mybir as mybir
    import concourse.tile as tile

    f32 = mybir.dt.float32
    bf16 = mybir.dt.bfloat16
    fp8 = mybir.dt.float8e4
    DR = mybir.MatmulPerfMode.DoubleRow

    nc = bacc.Bacc("TRN2", target_bir_lowering=False, debug=False,
                   enable_asserts=False)

    xb_d = nc.dram_tensor("xb", [P, NB, T], bf16, kind="ExternalInput").ap()
    xq_d = nc.dram_tensor("xq", [P, NPQ, 2, T], fp8, kind="ExternalInput").ap()
    wb_d = nc.dram_tensor("wb", [P, NB, D_OUT], bf16, kind="ExternalInput").ap()
    wq_d = nc.dram_tensor("wq", [P, NF, D_OUT], fp8, kind="ExternalInput").ap()
    ab_d = nc.dram_tensor("ab", [P, SPB, NB, RANK], bf16,
                          kind="ExternalInput").ap()
    aq_d = nc.dram_tensor("aq", [P, SPB, NPQ, 2, RANK], fp8,
                          kind="ExternalInput").ap()
    bt_d = nc.dram_tensor("bt", [RB, SPB, D_OUT], bf16,
                          kind="ExternalInput").ap()
    id_d = nc.dram_tensor("idm", [P, P], bf16, kind="ExternalInput").ap()
    out_d = nc.dram_tensor("out", [P, T // P, D_OUT], bf16,
                           kind="ExternalOutput").ap()

    with tile.TileContext(nc) as tc:
        with (
            tc.tile_pool(name="res", bufs=1) as res_pool,
            tc.tile_pool(name="xb", bufs=2 * NB) as xb_pool,
            tc.tile_pool(name="xq", bufs=2 * NPQ) as xq_pool,
            tc.tile_pool(name="wb", bufs=8) as wb_pool,
            tc.tile_pool(name="il", bufs=2) as il_pool,
            tc.tile_pool(name="il2", bufs=2) as il2_pool,
            tc.tile_pool(name="ob", bufs=8) as out_pool,
            tc.tile_pool(name="ps", bufs=7, space="PSUM") as ps_pool,
            tc.tile_pool(name="psil", bufs=1, space="PSUM") as psil_pool,
        ):
            # ---- resident tensors ----
            wq_sb = res_pool.tile([P, NF, D_OUT], fp8, name="wq", tag="wq")
            ab_sb = res_pool.tile([P, SPB, NB, RANK], bf16, name="ab",
                                  tag="ab")
            aq_sb = res_pool.tile([P, SPB, NPQ, 2, RANK], fp8, name="aq",
                                  tag="aq")
            bt_sb = res_pool.tile([RB, SPB, D_OUT], bf16, name="bt", tag="bt")
            id_sb = res_pool.tile([P, P], bf16, name="idm", tag="idm")

            xq_tiles = [None] * N_TB
            xb_tiles = [None] * N_TB
            il2_tiles = [None] * N_TB
            pre_w = {}     # (tb, o) -> list of preloaded W pair tiles

            def emit_block0_stream():
                """Block 0 startup: ~8MB must land in ~26us (o=0 pass is
                DMA-bound from cold start). Emit everything o=0 needs in
                need order, round-robin across the two HW-DGE queues
                (sync/scalar); bf16 x even-tiles ride gpsimd (SW DGE) in
                parallel. aq leads (DR riders run from step 0); id (for
                the epilogue transpose) goes last."""
                xq_tiles[0] = [None] * NPQ
                xb_tiles[0] = [None] * NB
                HWJ = 16       # xb tiles below this index ride the HW queues
                items = [("aq", 0, 49)]
                for p in range(NPQ):
                    items += [("xq", p, 128), ("wqc", p, 128)]
                w_tiles = []
                for j in range(NB):
                    if j % 2 == 0:
                        items.append(("w", j // 2, 256))
                    if j == 5:
                        items.append(("ab", 0, 425))
                    if j < HWJ:
                        items.append(("xb", j, 128))
                # tail of the stream: first 4 W pairs of pass o=1 (the
                # trace shows 4-6us gaps inside o=1 when its pairs queue
                # behind block-0 stream residue)
                items += [("idm", 0, 32)]
                w1_tiles = []
                for pg in range(4):
                    items.append(("w1", pg, 256))
                qb = [0, 0]    # greedy byte balance across sync/scalar
                for kind, idx, sz in items:
                    qi = 0 if qb[0] <= qb[1] else 1
                    qb[qi] += sz
                    eng = nc.sync if qi == 0 else nc.scalar
                    if kind == "aq":
                        eng.dma_start(aq_sb[:], aq_d)
                    elif kind == "ab":
                        eng.dma_start(ab_sb[:], ab_d)
                    elif kind == "idm":
                        eng.dma_start(id_sb[:], id_d)
                    elif kind == "xq":
                        t = xq_pool.tile([P, 2, TB], fp8,
                                         name=f"xq_0_{idx}", tag="xq")
                        eng.dma_start(t[:], xq_d[:, idx, :, 0:TB])
                        xq_tiles[0][idx] = t
                    elif kind == "wqc":
                        eng.dma_start(
                            wq_sb[:, 2 * idx:2 * idx + 2, 0:O_TILE],
                            wq_d[:, 2 * idx:2 * idx + 2, 0:O_TILE])
                    elif kind == "xb":
                        t = xb_pool.tile([P, TB], bf16,
                                         name=f"xb_0_{idx}", tag="xb")
                        eng.dma_start(t[:], xb_d[:, idx, 0:TB])
                        xb_tiles[0][idx] = t
                    elif kind == "w":
                        j = 2 * idx
                        w2 = wb_pool.tile([P, 2, O_TILE], bf16,
                                          name=f"w_0_0_{j}", tag="w")
                        eng.dma_start(w2[:], wb_d[:, j:j + 2, 0:O_TILE])
                        w_tiles.append(w2)
                    elif kind == "w1":
                        j = 2 * idx
                        w2 = wb_pool.tile([P, 2, O_TILE], bf16,
                                          name=f"w_0_1_{j}", tag="w")
                        eng.dma_start(
                            w2[:], wb_d[:, j:j + 2, O_TILE:2 * O_TILE])
                        w1_tiles.append(w2)
                for j in range(HWJ, NB):
                    t = xb_pool.tile([P, TB], bf16, name=f"xb_0_{j}",
                                     tag="xb")
                    nc.gpsimd.dma_start(t[:], xb_d[:, j, 0:TB])
                    xb_tiles[0][j] = t
                nc.gpsimd.dma_start(bt_sb[:], bt_d)
                pre_w[(0, 0)] = w_tiles
                pre_w[(0, 1)] = w1_tiles

            def emit_block_loads(tb):
                """x tiles for block tb>=1: stream on gpsimd during the
                previous block's ~200us of compute."""
                xq_tiles[tb] = []
                xb_tiles[tb] = []
                for p in range(NPQ):
                    t = xq_pool.tile([P, 2, TB], fp8, name=f"xq_{tb}_{p}",
                                     tag="xq")
                    nc.gpsimd.dma_start(
                        t[:], xq_d[:, p, :, tb * TB:(tb + 1) * TB])
                    xq_tiles[tb].append(t)
                for j in range(NB):
                    t = xb_pool.tile([P, TB], bf16, name=f"xb_{tb}_{j}",
                                     tag="xb")
                    nc.gpsimd.dma_start(t[:], xb_d[:, j, tb * TB:(tb + 1) * TB])
                    xb_tiles[tb].append(t)

            def emit_pass(tb, o, riders, rider_skip, bf16_first=False):
                """One o-pass: NPQ DoubleRow steps (resident fp8 W) and NB
                bf16 steps (W pairs streamed sync/scalar). DR steps lead by
                default (gives the W queues a breather at pass start);
                block0-o=0 runs bf16-first to match cold-start DMA arrival
                order. Returns the 4 psum tiles (caller finishes
                lora2/eviction)."""
                s = tb
                oc = slice(o * O_TILE, (o + 1) * O_TILE)
                psums = [ps_pool.tile([P, O_TILE], f32,
                                      name=f"ps_{tb}_{o}_{i}", tag="ps")
                         for i in range(TT)]
                state = {"r_idx": 0, "deferred": []}
                n_riders = NPQ + NB

                def dr_steps(first):
                    for p in range(NPQ):
                        xqt = xq_tiles[tb][p]
                        for tt in range(TT):
                            nc.tensor.matmul(
                                psums[tt][:],
                                xqt[:, :, tt * P:(tt + 1) * P],
                                wq_sb[:, 2 * p:2 * p + 2, oc],
                                start=(first and p == 0), stop=False,
                                perf_mode=DR)
                            if riders:
                                # start=False always: 4 interleaved
                                # accumulation groups share this bank at
                                # different column offsets, and a start=True
                                # zeroes beyond its own region (hw-observed);
                                # the bank is DVE-zeroed before the pass.
                                nc.tensor.matmul(
                                    ps_il[:, tt * RANK:(tt + 1) * RANK],
                                    xqt[:, :, tt * P:(tt + 1) * P],
                                    aq_sb[:, s, p],
                                    start=False,
                                    stop=(state["r_idx"] == n_riders - 1
                                          and not state["deferred"]),
                                    perf_mode=DR)
                        state["r_idx"] += 1

                def bf_steps(first):
                    w2 = None
                    for j in range(NB):
                        if j % 2 == 0:
                            pg = j // 2
                            pre = pre_w.get((tb, o))
                            if pre is not None and pg < len(pre):
                                w2 = pre[pg]
                            else:
                                w2 = wb_pool.tile(
                                    [P, 2, O_TILE], bf16,
                                    name=f"w_{tb}_{o}_{j}", tag="w")
                                eng = nc.sync if pg % 2 == 0 else nc.scalar
                                eng.dma_start(w2[:], wb_d[:, j:j + 2, oc])
                            # block 0: stream the next wq o-chunk late in the
                            # previous pass's pair stream (early injection
                            # delayed that pass's own pairs on the cold
                            # scalar queue - 4-5us gaps at the o=1/o=2
                            # boundaries in the trace)
                            if tb == 0 and o < N_OT - 1 and pg == 8:
                                oc2 = slice((o + 1) * O_TILE,
                                            (o + 2) * O_TILE)
                                nc.scalar.dma_start(wq_sb[:, :, oc2],
                                                    wq_d[:, :, oc2])
                        for tt in range(TT):
                            nc.tensor.matmul(
                                psums[tt][:],
                                xb_tiles[tb][j][:, tt * P:(tt + 1) * P],
                                w2[:, j % 2],
                                start=(first and j == 0), stop=False)
                            if riders:
                                if rider_skip and j < rider_skip:
                                    if tt == 0:
                                        state["deferred"].append(j)
                                    continue
                                nc.tensor.matmul(
                                    ps_il[:, tt * RANK:(tt + 1) * RANK],
                                    xb_tiles[tb][j][:, tt * P:(tt + 1) * P],
                                    ab_sb[:, s, j],
                                    start=False,
                                    stop=(state["r_idx"] == n_riders - 1
                                          and not state["deferred"]))
                        state["r_idx"] += 1

                if bf16_first:
                    bf_steps(True)
                    dr_steps(False)
                else:
                    dr_steps(True)
                    bf_steps(False)
                deferred = state["deferred"]
                # cleanup riders whose ab tile hadn't landed yet (block 0
                # start): re-load the stationary (costs an exposed LS each)
                for di, j in enumerate(deferred):
                    for tt in range(TT):
                        nc.tensor.matmul(
                            ps_il[:, tt * RANK:(tt + 1) * RANK],
                            xb_tiles[tb][j][:, tt * P:(tt + 1) * P],
                            ab_sb[:, s, j],
                            start=False, stop=(di == len(deferred) - 1))
                return psums

            def emit_lora2(tb, o, psums):
                s = tb
                oc = slice(o * O_TILE, (o + 1) * O_TILE)
                for tt in range(TT):
                    nc.tensor.matmul(
                        psums[tt][:],
                        il2_tiles[tb][:, tt * P:(tt + 1) * P],
                        bt_sb[:, s, oc],
                        start=False, stop=True)

            def emit_evict(tb, o, psums, final=False):
                for tt in range(TT):
                    o_t = out_pool.tile([P, O_TILE], bf16,
                                        name=f"o_{tb}_{o}_{tt}", tag="o")
                    if final and tt % 2 == 1:
                        # tail: split the last evictions across ACT + DVE
                        nc.scalar.copy(o_t[:], psums[tt][:])
                        nc.sync.dma_start(
                            out_d[:, tb * TT + tt,
                                  o * O_TILE:(o + 1) * O_TILE], o_t[:])
                    else:
                        nc.vector.tensor_copy(o_t[:], psums[tt][:])
                        nc.scalar.dma_start(
                            out_d[:, tb * TT + tt,
                                  o * O_TILE:(o + 1) * O_TILE], o_t[:])

            emit_block0_stream()

            for tb in range(N_TB):
                # rider psum: [tokens, tt*rank] — 9th psum region (1 bank)
                ps_il = psil_pool.tile([P, TT * RANK], f32,
                                       name=f"psil_{tb}", tag="psil")
                nc.vector.memset(ps_il[:], 0.0)

                # ---- o=0: base + riders; lora2 after the epilogue ----
                # rider_skip=0 even for block 0: ab lands ~15us on the HW
                # stream while the first bf16 step is x-arrival-bound to
                # ~24us, so riders never expose a stall
                psums0 = emit_pass(tb, 0, riders=True, rider_skip=0)
                # LoRA-1 epilogue: psum_il -> sbuf -> PE-transpose (via
                # identity) -> [17, 512] inter tile with ones row (bias)
                il_sb = il_pool.tile([P, TT * RANK], bf16,
                                     name=f"il_{tb}", tag="il")
                nc.vector.tensor_copy(il_sb[:], ps_il[:])
                tr_ps = psil_pool.tile([RANK, TT, P], bf16,
                                       name=f"tr_{tb}", tag="psil")
                for tt in range(TT):
                    nc.tensor.transpose(
                        tr_ps[:, tt, :],
                        il_sb[:, tt * RANK:(tt + 1) * RANK], id_sb[:])
                il2 = il2_pool.tile([RB, TB], bf16, name=f"il2_{tb}",
                                    tag="il2")
                nc.vector.memset(il2[:], 1.0)
                nc.vector.tensor_copy(il2[0:RANK, :], tr_ps[:])
                il2_tiles[tb] = il2

                emit_lora2(tb, 0, psums0)
                emit_evict(tb, 0, psums0)

                for o in range(1, N_OT):
                    psums = emit_pass(tb, o, riders=False, rider_skip=0)
                    if o == 1 and tb + 1 < N_TB:
                        emit_block_loads(tb + 1)
                    emit_lora2(tb, o, psums)
                    final = (tb == N_TB - 1 and o == N_OT - 1)
                    emit_evict(tb, o, psums, final=final)

    nc.compile()
    return nc


def _get_compiled():
    if "nc" not in _COMPILED:
        _COMPILED["nc"] = _build()
    return _COMPILED["nc"]


def kernel(x, adapter_ids, A_all, B_all, W, b):
    global LAST_RESULTS
    _ensure_axon_hooks_module()
    from concourse.bass_utils import run_bass_kernel_spmd
    from ml_dtypes import bfloat16, float8_e4m3fn

    x = np.asarray(x, dtype=np.float32)
    adapter_ids = np.asarray(adapter_ids)
    A_all = np.asarray(A_all, dtype=np.float32)
    B_all = np.asarray(B_all, dtype=np.float32)
    W = np.asarray(W, dtype=np.float32)
    b = np.asarray(b, dtype=np.float32)

    nc = _get_compiled()

    # ---- host-side layout prep (gather/scale/cast/transpose only) ----
    # W^T tiles: [p, k, o] = W[o, k*128+p]
    wt = np.ascontiguousarray(
        W.T.reshape(KT, P, D_OUT).transpose(1, 0, 2))        # (P, KT, D_OUT)
    A_batch = A_all[adapter_ids]                             # (B, R, D_IN)
    B_batch = B_all[adapter_ids] * SCALING                   # (B, D_OUT, R)

    idm = np.eye(P, dtype=np.float32).astype(bfloat16)

    in_maps = []
    for c in range(N_CORES):
        # stagger each core's o-axis so the SPMD cores don't all stream
        # the same W bytes at the same instant
        sh = (c % N_OT) * O_TILE
        wt_c = np.roll(wt, -sh, axis=2)
        wq_np = np.ascontiguousarray(wt_c[:, :NF, :] * SWQ).astype(
            float8_e4m3fn)
        wb_np = np.ascontiguousarray(wt_c[:, NF:, :]).astype(bfloat16)

        xs = x[c * SPB:(c + 1) * SPB].reshape(T, D_IN)
        xt = xs.reshape(T, KT, P).transpose(2, 1, 0)          # (P, KT, T)
        xq_np = np.ascontiguousarray(
            (xt[:, :NF, :] * SX).reshape(P, NPQ, 2, T)).astype(float8_e4m3fn)
        xb_np = np.ascontiguousarray(xt[:, NF:, :]).astype(bfloat16)

        A_c = A_batch[c * SPB:(c + 1) * SPB]                  # (SPB, R, D_IN)
        at = A_c.reshape(SPB, RANK, KT, P).transpose(3, 0, 2, 1)
        # (P, SPB, KT, R)
        aq_np = np.ascontiguousarray(
            (at[:, :, :NF, :] * SAQ).reshape(P, SPB, NPQ, 2, RANK)).astype(
            float8_e4m3fn)
        ab_np = np.ascontiguousarray(at[:, :, NF:, :]).astype(bfloat16)

        B_c = B_batch[c * SPB:(c + 1) * SPB]                  # (SPB, D_OUT, R)
        bt_base = np.roll(B_c.transpose(2, 0, 1), -sh, axis=2)  # (R, SPB, DO)
        bias_row = np.broadcast_to(np.roll(b, -sh), (1, SPB, D_OUT))
        bt_np = np.ascontiguousarray(
            np.concatenate([bt_base, bias_row], axis=0)).astype(bfloat16)

        in_maps.append({
            "xb": xb_np, "xq": xq_np, "wb": wb_np, "wq": wq_np,
            "ab": ab_np, "aq": aq_np, "bt": bt_np, "idm": idm,
        })

    res = run_bass_kernel_spmd(nc, in_maps, core_ids=list(range(N_CORES)))
    LAST_RESULTS = res

    out = np.empty((B_SZ, S, D_OUT), dtype=np.float32)
    for c in range(N_CORES):
        sh = (c % N_OT) * O_TILE
        oc = np.roll(res.results[c]["out"].astype(np.float32), sh, axis=2)
        out[c * SPB:(c + 1) * SPB] = (
            oc.transpose(1, 0, 2).reshape(T, D_OUT).reshape(SPB, S, D_OUT))
    return out


# revision 39
# speedup vs baseline: 1.1853x; 1.0021x over previous
"""Batched LoRA Linear on 8 Trainium2 NeuronCores (Bass/Tile).

Computes, for x (32, 512, 4096), adapter_ids (32,), A_all (32, 16, 4096),
B_all (32, 4096, 16), W (4096, 4096), b (4096,):

    out = x @ W.T + b + 2.0 * ((x @ A[aid].T) @ B[aid].T)

Sharding: data-parallel over batch — 4 samples per core; W/b replicated.

Per-core device kernel v2 (883.7us measured vs the 970.6us bf16
baseline, -9.0%; tensor busy ~830us, startup ~36us DMA-bound ramp,
tail ~11us):
 - T_BLOCK=512: one sample per block, 4 t-tiles per o-pass, 4 blocks.
 - Split-k fp8: the first NF=8 of 32 k-tiles run as fp8e4 DoubleRow
   matmuls (contraction 256/instr, 2x bf16 throughput, hw-measured
   214.8ns vs 222.0ns per 512-col instr). Product-1 scales
   (x*1/8, W*8) make the fp8 partial sums land in the SAME fp32 PSUM
   accumulation group as the bf16 k-tiles - no merge pass. Measured
   rel_fro 1.897e-2 (= numpy sim prediction; gate 2e-2; NF=6 gives
   1.649e-2 at +25us if more margin is ever needed).
 - LoRA-1 fused as 16-col "rider" matmuls after each base matmul,
   reusing its stationary x-tile (hw-measured +13ns per rider vs
   ~850ns/k-tile standalone). Riders accumulate inter^T (tokens x
   rank) in a 9th psum region; a PE-transpose epilogue (4x [128,16]
   transposes via identity) yields the [17, 512] inter tile (ones row
   carries the bias through LoRA-2 as a rank-1 term). The rider bank
   is DVE-zeroed and all riders use start=False: a start=True from
   one of the 4 interleaved per-tt groups sharing the bank zeroes
   beyond its own column range (hw-observed).
 - o=0's LoRA-2 waits for the transpose epilogue (~0.5us tensor stall
   per block), then accumulates into the still-open o=0 psums before
   eviction - every pass ends uniformly via LoRA-2 stop=True. PSUM =
   7 x [128,512] ps slots + 1 rider slot = exactly 8 banks; with 4
   allocations per pass the 7-slot rotation pairs each pass's LAST
   psum with the previous pass's FIRST-evicted bank (zero-stall).
 - W streamed as bf16 k-pairs alternating sync/scalar HW-DGE queues
   (pair cadence 1.7us/queue); fp8 W (3MB) resident in SBUF.
"""

import sys
import types

import numpy as np

# ---------------------------------------------------------------- constants
P = 128
B_SZ = 32            # batch
S = 512              # seq len
D_IN = 4096
D_OUT = 4096
RANK = 16
RB = RANK + 1        # + ones row: bias rides LoRA-2 as a rank-1 term
N_CORES = 8
SPB = B_SZ // N_CORES          # samples per core = 4
T = SPB * S                    # tokens per core = 2048
KT = D_IN // P                 # 32 k-tiles
TB = 512                       # tokens per block = one sample
N_TB = T // TB                 # 4 blocks
TT = TB // P                   # 4 t-tiles per block
O_TILE = 512
N_OT = D_OUT // O_TILE         # 8 o-tiles
SCALING = 2.0

NF = 8                         # fp8 k-tiles (even); rest bf16
NB = KT - NF                   # 26 bf16 k-tiles
NPQ = NF // 2                  # fp8 DoubleRow pairs per o-pass
SX = 0.125                     # x fp8 scale; SX*SWQ == 1 (exact, pow2)
SWQ = 8.0
SAQ = 8.0                      # A fp8 scale; SX*SAQ == 1

LAST_RESULTS = None            # test harness reads exec_time_ns from here

_COMPILED = {}


def _ensure_axon_hooks_module():
    try:
        import antenv.axon_hooks  # noqa: F401
        return
    except ImportError:
        pass
    try:
        import antenv
    except ImportError:
        return
    mod = types.ModuleType("antenv.axon_hooks")
    state = {"hook": None}
    mod.set_axon_ntff_profile_hook = lambda h: state.__setitem__("hook", h)
    mod.get_axon_ntff_profile_hook = lambda: state["hook"]
    sys.modules["antenv.axon_hooks"] = mod
    antenv.axon_hooks = mod


def _build():
    import concourse.bacc as bacc
    import concourse.bass as bass
    import concourse.mybir as mybir
    import concourse.tile as tile

    f32 = mybir.dt.float32
    bf16 = mybir.dt.bfloat16
    fp8 = mybir.dt.float8e4
    DR = mybir.MatmulPerfMode.DoubleRow

    nc = bacc.Bacc("TRN2", target_bir_lowering=False, debug=False,
                   enable_asserts=False)

    xb_d = nc.dram_tensor("xb", [P, NB, T], bf16, kind="ExternalInput").ap()
    xq_d = nc.dram_tensor("xq", [P, NPQ, 2, T], fp8, kind="ExternalInput").ap()
    wb_d = nc.dram_tensor("wb", [P, NB, D_OUT], bf16, kind="ExternalInput").ap()
    wq_d = nc.dram_tensor("wq", [P, NF, D_OUT], fp8, kind="ExternalInput").ap()
    ab_d = nc.dram_tensor("ab", [P, SPB, NB, RANK], bf16,
                          kind="ExternalInput").ap()
    aq_d = nc.dram_tensor("aq", [P, SPB, NPQ, 2, RANK], fp8,
                          kind="ExternalInput").ap()
    bt_d = nc.dram_tensor("bt", [RB, SPB, D_OUT], bf16,
                          kind="ExternalInput").ap()
    id_d = nc.dram_tensor("idm", [P, P], bf16, kind="ExternalInput").ap()
    out_d = nc.dram_tensor("out", [P, T // P, D_OUT], bf16,
                           kind="ExternalOutput").ap()

    with tile.TileContext(nc) as tc:
        with (
            tc.tile_pool(name="res", bufs=1) as res_pool,
            tc.tile_pool(name="xb", bufs=2 * NB) as xb_pool,
            tc.tile_pool(name="xq", bufs=2 * NPQ) as xq_pool,
            tc.tile_pool(name="wb", bufs=8) as wb_pool,
            tc.tile_pool(name="il", bufs=2) as il_pool,
            tc.tile_pool(name="il2", bufs=2) as il2_pool,
            tc.tile_pool(name="ob", bufs=8) as out_pool,
            tc.tile_pool(name="ps", bufs=7, space="PSUM") as ps_pool,
            tc.tile_pool(name="psil", bufs=1, space="PSUM") as psil_pool,
        ):
            # ---- resident tensors ----
            wq_sb = res_pool.tile([P, NF, D_OUT], fp8, name="wq", tag="wq")
            ab_sb = res_pool.tile([P, SPB, NB, RANK], bf16, name="ab",
                                  tag="ab")
            aq_sb = res_pool.tile([P, SPB, NPQ, 2, RANK], fp8, name="aq",
                                  tag="aq")
            bt_sb = res_pool.tile([RB, SPB, D_OUT], bf16, name="bt", tag="bt")
            id_sb = res_pool.tile([P, P], bf16, name="idm", tag="idm")

            xq_tiles = [None] * N_TB
            xb_tiles = [None] * N_TB
            il2_tiles = [None] * N_TB
            pre_w = {}     # (tb, o) -> list of preloaded W pair tiles

            def emit_block0_stream():
                """Block 0 startup: ~8MB must land in ~26us (o=0 pass is
                DMA-bound from cold start). Emit everything o=0 needs in
                need order, round-robin across the two HW-DGE queues
                (sync/scalar); bf16 x even-tiles ride gpsimd (SW DGE) in
                parallel. aq leads (DR riders run from step 0); id (for
                the epilogue transpose) goes last."""
                xq_tiles[0] = [None] * NPQ
                xb_tiles[0] = [None] * NB
                HWJ = 12       # xb tiles below this index ride the HW queues
                               # (j>=12 needed ~31us+, by then SW-DGE is warm;
                               # keeping them off the HW queues lightens the
                               # cold-start critical stream by 0.5MB)
                items = [("aq", 0, 49)]
                for p in range(NPQ):
                    items += [("xq", p, 128), ("wqc", p, 128)]
                w_tiles = []
                for j in range(NB):
                    if j % 2 == 0:
                        items.append(("w", j // 2, 256))
                    if j == 5:
                        items.append(("ab", 0, 425))
                    if j < HWJ:
                        items.append(("xb", j, 128))
                # tail of the stream: first 4 W pairs of pass o=1 (the
                # trace shows 4-6us gaps inside o=1 when its pairs queue
                # behind block-0 stream residue)
                items += [("idm", 0, 32)]
                w1_tiles = []
                for pg in range(4):
                    items.append(("w1", pg, 256))
                qb = [0, 0]    # greedy byte balance across sync/scalar
                for kind, idx, sz in items:
                    qi = 0 if qb[0] <= qb[1] else 1
                    qb[qi] += sz
                    eng = nc.sync if qi == 0 else nc.scalar
                    if kind == "aq":
                        eng.dma_start(aq_sb[:], aq_d)
                    elif kind == "ab":
                        eng.dma_start(ab_sb[:], ab_d)
                    elif kind == "idm":
                        eng.dma_start(id_sb[:], id_d)
                    elif kind == "xq":
                        t = xq_pool.tile([P, 2, TB], fp8,
                                         name=f"xq_0_{idx}", tag="xq")
                        eng.dma_start(t[:], xq_d[:, idx, :, 0:TB])
                        xq_tiles[0][idx] = t
                    elif kind == "wqc":
                        eng.dma_start(
                            wq_sb[:, 2 * idx:2 * idx + 2, 0:O_TILE],
                            wq_d[:, 2 * idx:2 * idx + 2, 0:O_TILE])
                    elif kind == "xb":
                        t = xb_pool.tile([P, TB], bf16,
                                         name=f"xb_0_{idx}", tag="xb")
                        eng.dma_start(t[:], xb_d[:, idx, 0:TB])
                        xb_tiles[0][idx] = t
                    elif kind == "w":
                        j = 2 * idx
                        w2 = wb_pool.tile([P, 2, O_TILE], bf16,
                                          name=f"w_0_0_{j}", tag="w")
                        eng.dma_start(w2[:], wb_d[:, j:j + 2, 0:O_TILE])
                        w_tiles.append(w2)
                    elif kind == "w1":
                        j = 2 * idx
                        w2 = wb_pool.tile([P, 2, O_TILE], bf16,
                                          name=f"w_0_1_{j}", tag="w")
                        eng.dma_start(
                            w2[:], wb_d[:, j:j + 2, O_TILE:2 * O_TILE])
                        w1_tiles.append(w2)
                for j in range(HWJ, NB):
                    t = xb_pool.tile([P, TB], bf16, name=f"xb_0_{j}",
                                     tag="xb")
                    nc.gpsimd.dma_start(t[:], xb_d[:, j, 0:TB])
                    xb_tiles[0][j] = t
                nc.gpsimd.dma_start(bt_sb[:], bt_d)
                pre_w[(0, 0)] = w_tiles
                pre_w[(0, 1)] = w1_tiles

            def emit_block_loads(tb):
                """x tiles for block tb>=1: stream on gpsimd during the
                previous block's ~200us of compute."""
                xq_tiles[tb] = []
                xb_tiles[tb] = []
                for p in range(NPQ):
                    t = xq_pool.tile([P, 2, TB], fp8, name=f"xq_{tb}_{p}",
                                     tag="xq")
                    nc.gpsimd.dma_start(
                        t[:], xq_d[:, p, :, tb * TB:(tb + 1) * TB])
                    xq_tiles[tb].append(t)
                for j in range(NB):
                    t = xb_pool.tile([P, TB], bf16, name=f"xb_{tb}_{j}",
                                     tag="xb")
                    nc.gpsimd.dma_start(t[:], xb_d[:, j, tb * TB:(tb + 1) * TB])
                    xb_tiles[tb].append(t)

            def emit_pass(tb, o, riders, rider_skip, bf16_first=False):
                """One o-pass: NPQ DoubleRow steps (resident fp8 W) and NB
                bf16 steps (W pairs streamed sync/scalar). DR steps lead by
                default (gives the W queues a breather at pass start);
                block0-o=0 runs bf16-first to match cold-start DMA arrival
                order. Returns the 4 psum tiles (caller finishes
                lora2/eviction)."""
                s = tb
                oc = slice(o * O_TILE, (o + 1) * O_TILE)
                psums = [ps_pool.tile([P, O_TILE], f32,
                                      name=f"ps_{tb}_{o}_{i}", tag="ps")
                         for i in range(TT)]
                state = {"r_idx": 0, "deferred": []}
                n_riders = NPQ + NB

                def dr_steps(first):
                    for p in range(NPQ):
                        xqt = xq_tiles[tb][p]
                        for tt in range(TT):
                            nc.tensor.matmul(
                                psums[tt][:],
                                xqt[:, :, tt * P:(tt + 1) * P],
                                wq_sb[:, 2 * p:2 * p + 2, oc],
                                start=(first and p == 0), stop=False,
                                perf_mode=DR)
                            if riders:
                                # start=False always: 4 interleaved
                                # accumulation groups share this bank at
                                # different column offsets, and a start=True
                                # zeroes beyond its own region (hw-observed);
                                # the bank is DVE-zeroed before the pass.
                                nc.tensor.matmul(
                                    ps_il[:, tt * RANK:(tt + 1) * RANK],
                                    xqt[:, :, tt * P:(tt + 1) * P],
                                    aq_sb[:, s, p],
                                    start=False,
                                    stop=(state["r_idx"] == n_riders - 1
                                          and not state["deferred"]),
                                    perf_mode=DR)
                        state["r_idx"] += 1

                def bf_steps(first):
                    w2 = None
                    for j in range(NB):
                        if j % 2 == 0:
                            pg = j // 2
                            pre = pre_w.get((tb, o))
                            if pre is not None and pg < len(pre):
                                w2 = pre[pg]
                            else:
                                w2 = wb_pool.tile(
                                    [P, 2, O_TILE], bf16,
                                    name=f"w_{tb}_{o}_{j}", tag="w")
                                eng = nc.sync if pg % 2 == 0 else nc.scalar
                                eng.dma_start(w2[:], wb_d[:, j:j + 2, oc])
                            # block 0: stream the next wq o-chunk late in the
                            # previous pass's pair stream (early injection
                            # delayed that pass's own pairs on the cold
                            # scalar queue - 4-5us gaps at the o=1/o=2
                            # boundaries in the trace)
                            if tb == 0 and o < N_OT - 1 and pg == 8:
                                oc2 = slice((o + 1) * O_TILE,
                                            (o + 2) * O_TILE)
                                nc.scalar.dma_start(wq_sb[:, :, oc2],
                                                    wq_d[:, :, oc2])
                        for tt in range(TT):
                            nc.tensor.matmul(
                                psums[tt][:],
                                xb_tiles[tb][j][:, tt * P:(tt + 1) * P],
                                w2[:, j % 2],
                                start=(first and j == 0), stop=False)
                            if riders:
                                if rider_skip and j < rider_skip:
                                    if tt == 0:
                                        state["deferred"].append(j)
                                    continue
                                nc.tensor.matmul(
                                    ps_il[:, tt * RANK:(tt + 1) * RANK],
                                    xb_tiles[tb][j][:, tt * P:(tt + 1) * P],
                                    ab_sb[:, s, j],
                                    start=False,
                                    stop=(state["r_idx"] == n_riders - 1
                                          and not state["deferred"]))
                        state["r_idx"] += 1

                if bf16_first:
                    bf_steps(True)
                    dr_steps(False)
                else:
                    dr_steps(True)
                    bf_steps(False)
                deferred = state["deferred"]
                # cleanup riders whose ab tile hadn't landed yet (block 0
                # start): re-load the stationary (costs an exposed LS each)
                for di, j in enumerate(deferred):
                    for tt in range(TT):
                        nc.tensor.matmul(
                            ps_il[:, tt * RANK:(tt + 1) * RANK],
                            xb_tiles[tb][j][:, tt * P:(tt + 1) * P],
                            ab_sb[:, s, j],
                            start=False, stop=(di == len(deferred) - 1))
                return psums

            def emit_lora2(tb, o, psums):
                s = tb
                oc = slice(o * O_TILE, (o + 1) * O_TILE)
                for tt in range(TT):
                    nc.tensor.matmul(
                        psums[tt][:],
                        il2_tiles[tb][:, tt * P:(tt + 1) * P],
                        bt_sb[:, s, oc],
                        start=False, stop=True)

            def emit_evict(tb, o, psums, final=False):
                for tt in range(TT):
                    o_t = out_pool.tile([P, O_TILE], bf16,
                                        name=f"o_{tb}_{o}_{tt}", tag="o")
                    if final and tt % 2 == 1:
                        # tail: split the last evictions across ACT + DVE
                        nc.scalar.copy(o_t[:], psums[tt][:])
                        nc.sync.dma_start(
                            out_d[:, tb * TT + tt,
                                  o * O_TILE:(o + 1) * O_TILE], o_t[:])
                    else:
                        nc.vector.tensor_copy(o_t[:], psums[tt][:])
                        nc.scalar.dma_start(
                            out_d[:, tb * TT + tt,
                                  o * O_TILE:(o + 1) * O_TILE], o_t[:])

            emit_block0_stream()

            for tb in range(N_TB):
                # rider psum: [tokens, tt*rank] — 9th psum region (1 bank)
                ps_il = psil_pool.tile([P, TT * RANK], f32,
                                       name=f"psil_{tb}", tag="psil")
                nc.vector.memset(ps_il[:], 0.0)

                # ---- o=0: base + riders; lora2 after the epilogue ----
                # rider_skip=0 even for block 0: ab lands ~15us on the HW
                # stream while the first bf16 step is x-arrival-bound to
                # ~24us, so riders never expose a stall
                psums0 = emit_pass(tb, 0, riders=True, rider_skip=0)
                # LoRA-1 epilogue: psum_il -> sbuf -> PE-transpose (via
                # identity) -> [17, 512] inter tile with ones row (bias)
                il_sb = il_pool.tile([P, TT * RANK], bf16,
                                     name=f"il_{tb}", tag="il")
                nc.vector.tensor_copy(il_sb[:], ps_il[:])
                tr_ps = psil_pool.tile([RANK, TT, P], bf16,
                                       name=f"tr_{tb}", tag="psil")
                for tt in range(TT):
                    nc.tensor.transpose(
                        tr_ps[:, tt, :],
                        il_sb[:, tt * RANK:(tt + 1) * RANK], id_sb[:])
                il2 = il2_pool.tile([RB, TB], bf16, name=f"il2_{tb}",
                                    tag="il2")
                nc.vector.memset(il2[:], 1.0)
                nc.vector.tensor_copy(il2[0:RANK, :], tr_ps[:])
                il2_tiles[tb] = il2

                emit_lora2(tb, 0, psums0)
                emit_evict(tb, 0, psums0)

                for o in range(1, N_OT):
                    psums = emit_pass(tb, o, riders=False, rider_skip=0)
                    if o == 1 and tb + 1 < N_TB:
                        emit_block_loads(tb + 1)
                    emit_lora2(tb, o, psums)
                    final = (tb == N_TB - 1 and o == N_OT - 1)
                    emit_evict(tb, o, psums, final=final)

    nc.compile()
    return nc


def _get_compiled():
    if "nc" not in _COMPILED:
        _COMPILED["nc"] = _build()
    return _COMPILED["nc"]


def kernel(x, adapter_ids, A_all, B_all, W, b):
    global LAST_RESULTS
    _ensure_axon_hooks_module()
    from concourse.bass_utils import run_bass_kernel_spmd
    from ml_dtypes import bfloat16, float8_e4m3fn

    x = np.asarray(x, dtype=np.float32)
    adapter_ids = np.asarray(adapter_ids)
    A_all = np.asarray(A_all, dtype=np.float32)
    B_all = np.asarray(B_all, dtype=np.float32)
    W = np.asarray(W, dtype=np.float32)
    b = np.asarray(b, dtype=np.float32)

    nc = _get_compiled()

    # ---- host-side layout prep (gather/scale/cast/transpose only) ----
    # W^T tiles: [p, k, o] = W[o, k*128+p]
    wt = np.ascontiguousarray(
        W.T.reshape(KT, P, D_OUT).transpose(1, 0, 2))        # (P, KT, D_OUT)
    A_batch = A_all[adapter_ids]                             # (B, R, D_IN)
    B_batch = B_all[adapter_ids] * SCALING                   # (B, D_OUT, R)

    idm = np.eye(P, dtype=np.float32).astype(bfloat16)

    in_maps = []
    for c in range(N_CORES):
        # stagger each core's o-axis so the SPMD cores don't all stream
        # the same W bytes at the same instant
        sh = (c % N_OT) * O_TILE
        wt_c = np.roll(wt, -sh, axis=2)
        wq_np = np.ascontiguousarray(wt_c[:, :NF, :] * SWQ).astype(
            float8_e4m3fn)
        wb_np = np.ascontiguousarray(wt_c[:, NF:, :]).astype(bfloat16)

        xs = x[c * SPB:(c + 1) * SPB].reshape(T, D_IN)
        xt = xs.reshape(T, KT, P).transpose(2, 1, 0)          # (P, KT, T)
        xq_np = np.ascontiguousarray(
            (xt[:, :NF, :] * SX).reshape(P, NPQ, 2, T)).astype(float8_e4m3fn)
        xb_np = np.ascontiguousarray(xt[:, NF:, :]).astype(bfloat16)

        A_c = A_batch[c * SPB:(c + 1) * SPB]                  # (SPB, R, D_IN)
        at = A_c.reshape(SPB, RANK, KT, P).transpose(3, 0, 2, 1)
        # (P, SPB, KT, R)
        aq_np = np.ascontiguousarray(
            (at[:, :, :NF, :] * SAQ).reshape(P, SPB, NPQ, 2, RANK)).astype(
            float8_e4m3fn)
        ab_np = np.ascontiguousarray(at[:, :, NF:, :]).astype(bfloat16)

        B_c = B_batch[c * SPB:(c + 1) * SPB]                  # (SPB, D_OUT, R)
        bt_base = np.roll(B_c.transpose(2, 0, 1), -sh, axis=2)  # (R, SPB, DO)
        bias_row = np.broadcast_to(np.roll(b, -sh), (1, SPB, D_OUT))
        bt_np = np.ascontiguousarray(
            np.concatenate([bt_base, bias_row], axis=0)).astype(bfloat16)

        in_maps.append({
            "xb": xb_np, "xq": xq_np, "wb": wb_np, "wq": wq_np,
            "ab": ab_np, "aq": aq_np, "bt": bt_np, "idm": idm,
        })

    res = run_bass_kernel_spmd(nc, in_maps, core_ids=list(range(N_CORES)))
    LAST_RESULTS = res

    out = np.empty((B_SZ, S, D_OUT), dtype=np.float32)
    for c in range(N_CORES):
        sh = (c % N_OT) * O_TILE
        oc = np.roll(res.results[c]["out"].astype(np.float32), sh, axis=2)
        out[c * SPB:(c + 1) * SPB] = (
            oc.transpose(1, 0, 2).reshape(T, D_OUT).reshape(SPB, S, D_OUT))
    return out
